# revision 10
# baseline (speedup 1.0000x reference)
"""Trainium2 Bass kernel: multi-head attention (B=4, S=2048, E=1024, H=16, D=64).

Sharding: 8 cores = 4 batches x 2 head-groups (8 heads each). Each core
computes attention for its (batch, 8-head group) and a partial output
projection over its 512 channels; the host sums the two partials per batch
and adds the output bias.

v2 design (vs v1 baseline at ~355us):
- Heads processed in pairs: head A occupies SBUF partitions 0-63, head B
  64-127.  QK^T score matmuls are K=64 row-tiled pairs (tile_position
  (0,0)/(64,0)) that execute CONCURRENTLY on the PE array -> ~1.7x faster
  scores (measured in micro-benchmark).
- Q/K projections pair-packed with block-diagonal weights (K=128 full
  array); biases added on the PSUM->SBUF copy (DVE tensor_scalar_add).
- exp() split between the Scalar engine (table exp, ~9/16 of tiles) and
  the Vector engine (Schraudolph-style bit-trick exp producing bf16 via
  uint16 bitcast, ~7/16) -- the Scalar engine alone (306us busy) was the
  baseline co-bottleneck.
- attnV: V_aug stationary (65 cols, ones column -> softmax denominator
  in row 64 of the accumulator).
- Normalization (reciprocal bcast + multiply) per (pair, s-block) on
  GPSIMD, recip broadcast via DRAM bounce.
- Output projection per s-block, overlapped with the next s-block's
  attention; result DMA'd straight from SBUF copies.
"""

import sys

sys.path.insert(0, "/opt/trn_rl_repo")

import numpy as np
import ml_dtypes

BF16 = ml_dtypes.bfloat16

B, S, E, H = 4, 2048, 1024, 16
D = E // H          # 64
HL = 8              # heads per core
NP = 4              # head pairs per core
N_CORES = 8
SB = 512            # s-block (psum bank) width

# DVE bit-exp: bf16 pattern of exp(l/8) ~= uint16(l * A + B)
A_EXP = float((2 ** 7 / np.log(2)) * 0.125)
B_EXP = float(127 * 128 - 7.0)

# which tt (0..15) of a (pair, s-block) slot compute exp on DVE
# (slot boundary tts 15/0/1 stay on ACT so DVE is free for the av release)
DVE_TT_STEADY = (2, 4, 5, 6, 8, 10, 12)
DVE_TT_SLOT0 = (8, 12)
DVE_TT_SLOT1 = (4, 6, 8, 10, 12)
PIPE_DELAY = 4      # attnV trails scores/exp by this many tt steps

_CACHE = {}


def build_nc():
    import concourse.mybir as mybir
    import concourse.tile as tile
    from concourse import bacc

    f32 = mybir.dt.float32
    bf16 = mybir.dt.bfloat16
    u16 = mybir.dt.uint16
    ADD = mybir.AluOpType.add
    MULT = mybir.AluOpType.mult

    n_tt = S // 128     # 16 t tiles
    n_sb = S // SB      # 4 s-blocks
    n_sc = S // 128     # 16 proj row tiles

    nc = bacc.Bacc(None)

    xp_d = nc.dram_tensor("xp", [NP, 128, S], bf16, kind="ExternalInput")
    wq_d = nc.dram_tensor("wq", [NP, 128, 128], bf16, kind="ExternalInput")
    wk_d = nc.dram_tensor("wk", [NP, 128, 128], bf16, kind="ExternalInput")
    wv_d = nc.dram_tensor("wv", [NP, 128, D + 1], bf16, kind="ExternalInput")
    bq_d = nc.dram_tensor("bq", [NP, 128, 1], f32, kind="ExternalInput")
    bk_d = nc.dram_tensor("bk", [NP, 128, 1], f32, kind="ExternalInput")
    bvb_d = nc.dram_tensor("bvb", [NP, 2, 4 * (D + 1)], bf16,
                           kind="ExternalInput")
    wot_d = nc.dram_tensor("wot", [HL * D, E], bf16, kind="ExternalInput")
    out_d = nc.dram_tensor("out", [S, E], f32, kind="ExternalOutput")
    recd = nc.dram_tensor("recd", [n_sb * NP, 2 * SB], f32)

    with tile.TileContext(nc) as tc:
        with (
            tc.tile_pool(name="xp", bufs=NP) as xp_pool,
            tc.tile_pool(name="w", bufs=3 * NP) as w_pool,
            tc.tile_pool(name="bias", bufs=4 * NP) as bias_pool,
            tc.tile_pool(name="qt", bufs=2 * NP) as qt_pool,
            tc.tile_pool(name="v", bufs=HL) as v_pool,
            tc.tile_pool(name="wot", bufs=4) as wot_pool,
            tc.tile_pool(name="et", bufs=7) as et_pool,
            tc.tile_pool(name="ct", bufs=NP) as ct_pool,
            tc.tile_pool(name="rel", bufs=6) as rel_pool,
            tc.tile_pool(name="bc", bufs=6) as bc_pool,
            tc.tile_pool(name="csrc", bufs=4) as cs_pool,
            tc.tile_pool(name="ctmp", bufs=2) as ctmp_pool,
            tc.tile_pool(name="osb", bufs=3) as osb_pool,
            tc.tile_pool(name="ps", bufs=2, space="PSUM") as ps_pool,
            tc.tile_pool(name="av", bufs=2, space="PSUM") as av_pool,
        ):
            # persistent tiles
            xps, qts, kts = [], [], []
            wqs, wks, wvs, bqs, bks, bvbs = [], [], [], [], [], []
            vss = []
            ct2s = []
            for p in range(NP):
                xps.append(xp_pool.tile([128, S], bf16, tag="xp",
                                        name=f"xp{p}"))
                qts.append(qt_pool.tile([128, S], bf16, tag="qt",
                                        name=f"qt{p}"))
                kts.append(qt_pool.tile([128, S], bf16, tag="qt",
                                        name=f"kt{p}"))
                wqs.append(w_pool.tile([128, 128], bf16, tag="w",
                                       name=f"wq{p}"))
                wks.append(w_pool.tile([128, 128], bf16, tag="w",
                                       name=f"wk{p}"))
                wvs.append(w_pool.tile([128, D + 1], bf16, tag="w",
                                       name=f"wv{p}"))
                bqs.append(bias_pool.tile([128, 1], f32, tag="bias",
                                          name=f"bq{p}"))
                bks.append(bias_pool.tile([128, 1], f32, tag="bias",
                                          name=f"bk{p}"))
                bvbs.append([bias_pool.tile([128, 4 * (D + 1)], bf16,
                                            tag="bias", name=f"bvb{p}_{h}")
                             for h in range(2)])
                vss.append([v_pool.tile([128, n_tt * (D + 1)], bf16, tag="v",
                                        name=f"vs{p}_{h}") for h in range(2)])
                ct2s.append(ct_pool.tile([128, S], bf16, tag="ct",
                                         name=f"ct{p}"))
            for p in range(NP):
                nc.sync.dma_start(out=xps[p][:, :], in_=xp_d[p])
                nc.scalar.dma_start(out=wqs[p][:, :], in_=wq_d[p])
                nc.scalar.dma_start(out=wks[p][:, :], in_=wk_d[p])
                nc.scalar.dma_start(out=wvs[p][:, :], in_=wv_d[p])
                nc.scalar.dma_start(out=bqs[p][:, :], in_=bq_d[p])
                nc.scalar.dma_start(out=bks[p][:, :], in_=bk_d[p])
                for h in range(2):
                    nc.scalar.dma_start(
                        out=bvbs[p][h][:, :],
                        in_=bvb_d[p, h].unsqueeze(0).broadcast_to(
                            (128, 4 * (D + 1))),
                    )
            wots = []
            for p in range(4):
                wt = wot_pool.tile([128, E], bf16, tag="wot", name=f"wot{p}")
                nc.sync.dma_start(out=wt[:, :],
                                  in_=wot_d[p * 128:(p + 1) * 128, :])
                wots.append(wt)

            def emit_qkv(p):
                # Q/K pair-packed projections, biases on the copy
                for which, wt, bias, dst in (
                    (0, wqs[p], bqs[p], qts[p]),
                    (1, wks[p], bks[p], kts[p]),
                ):
                    for half in range(2):
                        hsl = slice(half * 1024, (half + 1) * 1024)
                        psq = ps_pool.tile([128, 1024], f32, tag="ps",
                                           name=f"qk{p}_{which}_{half}")
                        for q4 in range(2):
                            qsl = slice(half * 1024 + q4 * 512,
                                        half * 1024 + (q4 + 1) * 512)
                            nc.tensor.matmul(psq[:, q4 * 512:(q4 + 1) * 512],
                                             wt[:, :], xps[p][:, qsl])
                        nc.vector.tensor_scalar_add(dst[:, hsl], psq[:, :],
                                                    bias[:, :])
                # V projections: row-tiled head pairs, 4 t-tiles per bank
                for tg in range(4):
                    psv = [ps_pool.tile([128, 1024], f32, tag="ps",
                                        name=f"v{p}_{tg}_{h}")
                           for h in range(2)]
                    for j in range(4):
                        tsl = slice((tg * 4 + j) * 128, (tg * 4 + j + 1) * 128)
                        csl = slice(j * (D + 1), (j + 1) * (D + 1))
                        nc.tensor.matmul(psv[0][:, csl], xps[p][0:64, tsl],
                                         wvs[p][0:64, :])
                        nc.tensor.matmul(psv[1][:, csl], xps[p][64:128, tsl],
                                         wvs[p][64:128, :])
                    gsl = slice(tg * 4 * (D + 1), (tg + 1) * 4 * (D + 1))
                    for h in range(2):
                        nc.vector.tensor_tensor(vss[p][h][:, gsl],
                                                psv[h][:, 0:4 * (D + 1)],
                                                bvbs[p][h][:, :], ADD)

            # slot state shared between the pipelined closures
            slot_av = {}

            def emit_step(p, sb, slot_idx, tt):
                """scores + exp for one tt of slot (p, sb)."""
                ssl = slice(sb * SB, (sb + 1) * SB)
                if slot_idx == 0:
                    dve_tt = DVE_TT_SLOT0
                elif slot_idx == 1:
                    dve_tt = DVE_TT_SLOT1
                else:
                    dve_tt = DVE_TT_STEADY
                tsl = slice(tt * 128, (tt + 1) * 128)
                ps = ps_pool.tile([128, 2 * SB], f32, tag="ps",
                                  name=f"sc{slot_idx}_{tt}")
                nc.tensor.matmul(ps[:, 0:SB], kts[p][0:64, tsl],
                                 qts[p][0:64, ssl])
                nc.tensor.matmul(ps[:, SB:2 * SB], kts[p][64:128, tsl],
                                 qts[p][64:128, ssl])
                et = et_pool.tile([128, 2 * SB], bf16, tag="et",
                                  name=f"et{slot_idx}_{tt}")
                if tt in dve_tt:
                    nc.vector.tensor_scalar(et[:, :].bitcast(u16),
                                            ps[:, :], A_EXP, B_EXP,
                                            MULT, ADD)
                else:
                    nc.scalar.activation(
                        et[:, :], ps[:, :],
                        mybir.ActivationFunctionType.Exp, scale=0.125)
                return et

            def emit_attnv(p, sb, slot_idx, tt, et):
                if tt == 0:
                    slot_av[slot_idx] = av_pool.tile(
                        [D + 1, 2 * SB], f32, tag="av", name=f"av{slot_idx}")
                av2 = slot_av[slot_idx]
                vsl = slice(tt * (D + 1), (tt + 1) * (D + 1))
                nc.tensor.matmul(av2[:, 0:SB], vss[p][0][:, vsl],
                                 et[:, 0:SB],
                                 start=(tt == 0), stop=(tt == n_tt - 1))
                nc.tensor.matmul(av2[:, SB:2 * SB], vss[p][1][:, vsl],
                                 et[:, SB:2 * SB],
                                 start=(tt == 0), stop=(tt == n_tt - 1))

            def emit_norm(p, sb, slot_idx):
                av2 = slot_av.pop(slot_idx)
                ssl = slice(sb * SB, (sb + 1) * SB)
                rel = [rel_pool.tile([D + 1, SB], bf16, tag="rel",
                                     name=f"rel{slot_idx}_{h}")
                       for h in range(2)]
                nc.vector.tensor_copy(rel[0][:, :], av2[:, 0:SB])
                nc.vector.tensor_copy(rel[1][:, :], av2[:, SB:2 * SB])
                cs = cs_pool.tile([128, 8], bf16, tag="csrc",
                                  name=f"cs{slot_idx}")
                nc.sync.dma_start(out=cs[:, 0:4], in_=rel[0][D:D + 1, :])
                nc.sync.dma_start(out=cs[:, 4:8], in_=rel[1][D:D + 1, :])
                rc = cs_pool.tile([128, 8], f32, tag="csrc",
                                  name=f"rc{slot_idx}")
                nc.vector.reciprocal(rc[:, :], cs[:, :])
                nc.sync.dma_start(out=recd[slot_idx, 0:SB], in_=rc[:, 0:4])
                nc.sync.dma_start(out=recd[slot_idx, SB:2 * SB],
                                  in_=rc[:, 4:8])
                bcs = []
                for h in range(2):
                    bc = bc_pool.tile([D, SB], f32, tag="bc",
                                      name=f"bc{slot_idx}_{h}")
                    nc.sync.dma_start(
                        out=bc[:, :],
                        in_=recd[slot_idx, h * SB:(h + 1) * SB]
                        .unsqueeze(0).broadcast_to((D, SB)))
                    bcs.append(bc)
                nc.gpsimd.tensor_tensor(ct2s[p][0:D, ssl], rel[0][0:D, :],
                                        bcs[0][:, :], MULT)
                ctmp = ctmp_pool.tile([D, SB], bf16, tag="ctmp",
                                      name=f"ctmp{slot_idx}")
                nc.gpsimd.tensor_tensor(ctmp[:, :], rel[1][0:D, :],
                                        bcs[1][:, :], MULT)
                nc.sync.dma_start(out=ct2s[p][D:2 * D, ssl], in_=ctmp[:, :])

            def emit_proj(sb):
                for sc in range(sb * 4, sb * 4 + 4):
                    csl = slice(sc * 128, (sc + 1) * 128)
                    pso = ps_pool.tile([128, 2 * SB], f32, tag="ps",
                                       name=f"pj{sc}")
                    for half in range(2):
                        hsl = slice(half * SB, (half + 1) * SB)
                        psl = pso[:, half * SB:(half + 1) * SB]
                        for p in range(NP):
                            nc.tensor.matmul(psl, ct2s[p][:, csl],
                                             wots[p][:, hsl],
                                             start=(p == 0), stop=(p == 3))
                    osb = osb_pool.tile([128, 2 * SB], f32, tag="osb",
                                        name=f"osb{sc}")
                    nc.vector.tensor_copy(osb[:, :], pso[:, :])
                    nc.scalar.dma_start(out=out_d[csl, :], in_=osb[:, :])

            # Global software pipeline: attnV (and trailing slot actions)
            # run PIPE_DELAY tt-steps behind scores/exp so the PE's
            # in-order queue never stalls waiting for an exp.
            from collections import deque
            deferred = deque()

            def run_deferred(n_keep):
                while len(deferred) > n_keep:
                    for fn in deferred.popleft():
                        fn()

            from functools import partial
            for sb in range(n_sb):
                for p in range(NP):
                    slot_idx = sb * NP + p
                    if sb == 0:
                        emit_qkv(p)
                    for tt in range(n_tt):
                        et = emit_step(p, sb, slot_idx, tt)
                        acts = [partial(emit_attnv, p, sb, slot_idx, tt, et)]
                        if tt == n_tt - 1:
                            acts.append(partial(emit_norm, p, sb, slot_idx))
                            if p == 0 and sb > 0:
                                acts.append(partial(emit_proj, sb - 1))
                        deferred.append(acts)
                        run_deferred(PIPE_DELAY)
            run_deferred(0)
            emit_proj(n_sb - 1)

    nc.compile()
    return nc


def prep_inputs(token_encodings, Wq, Wk, Wv, bq, bk, bv, Wo, bo):
    """Build per-core input maps. Core c = b*2+g."""
    x = np.asarray(token_encodings, dtype=np.float32)
    Wq = np.asarray(Wq, np.float32)
    Wk = np.asarray(Wk, np.float32)
    Wv = np.asarray(Wv, np.float32)
    bq = np.asarray(bq, np.float32)
    bk = np.asarray(bk, np.float32)
    bv = np.asarray(bv, np.float32)
    Wo = np.asarray(Wo, np.float32)
    maps = []
    for c in range(N_CORES):
        b, g = divmod(c, 2)
        xt_full = np.ascontiguousarray(x[b].T)  # (E, S)
        xp = np.zeros((NP, 128, S), dtype=BF16)
        wq_a = np.zeros((NP, 128, 128), dtype=BF16)
        wk_a = np.zeros((NP, 128, 128), dtype=BF16)
        wv_a = np.zeros((NP, 128, D + 1), dtype=BF16)
        bq_a = np.zeros((NP, 128, 1), dtype=np.float32)
        bk_a = np.zeros((NP, 128, 1), dtype=np.float32)
        bvb_a = np.zeros((NP, 2, 4 * (D + 1)), dtype=BF16)
        for p in range(NP):
            for h in range(2):
                hh = g * HL + 2 * p + h
                rsl = slice(h * 64, (h + 1) * 64)
                xp[p, rsl] = xt_full[hh * D:(hh + 1) * D].astype(BF16)
                wq_a[p, rsl, rsl] = Wq[hh].astype(BF16)
                wk_a[p, rsl, rsl] = Wk[hh].astype(BF16)
                wv_a[p, rsl, 0:D] = Wv[hh].astype(BF16)
                bq_a[p, rsl, 0] = bq[hh]
                bk_a[p, rsl, 0] = bk[hh]
                one_bv = np.concatenate([bv[hh], [1.0]]).astype(BF16)
                bvb_a[p, h] = np.tile(one_bv, 4)
        wot = np.ascontiguousarray(
            Wo[:, g * 512:(g + 1) * 512].T).astype(BF16)
        maps.append({"xp": xp, "wq": wq_a, "wk": wk_a, "wv": wv_a,
                     "bq": bq_a, "bk": bk_a, "bvb": bvb_a, "wot": wot})
    return maps


def kernel(**inputs):
    from concourse.bass_utils import run_bass_kernel_spmd

    if "nc" not in _CACHE:
        _CACHE["nc"] = build_nc()
    nc = _CACHE["nc"]
    in_maps = prep_inputs(**inputs)
    res = run_bass_kernel_spmd(nc, in_maps, list(range(N_CORES)))
    bo_f = np.asarray(inputs["bo"], np.float32)
    out = np.empty((B, S, E), dtype=np.float32)
    for b in range(B):
        out[b] = res.results[2 * b]["out"] + res.results[2 * b + 1]["out"] + bo_f
    return out


# revision 11
# speedup vs baseline: 1.0264x; 1.0264x over previous
"""Trainium2 Bass kernel: multi-head attention (B=4, S=2048, E=1024, H=16, D=64).

Sharding: 8 cores = 4 batches x 2 head-groups (8 heads each). Each core
computes attention for its (batch, 8-head group) and a partial output
projection over its 512 channels; the host sums the two partials per batch
and adds the output bias.

v2 design (vs v1 baseline at ~355us):
- Heads processed in pairs: head A occupies SBUF partitions 0-63, head B
  64-127.  QK^T score matmuls are K=64 row-tiled pairs (tile_position
  (0,0)/(64,0)) that execute CONCURRENTLY on the PE array -> ~1.7x faster
  scores (measured in micro-benchmark).
- Q/K projections pair-packed with block-diagonal weights (K=128 full
  array); biases added on the PSUM->SBUF copy (DVE tensor_scalar_add).
- exp() split between the Scalar engine (table exp, ~9/16 of tiles) and
  the Vector engine (Schraudolph-style bit-trick exp producing bf16 via
  uint16 bitcast, ~7/16) -- the Scalar engine alone (306us busy) was the
  baseline co-bottleneck.
- attnV: V_aug stationary (65 cols, ones column -> softmax denominator
  in row 64 of the accumulator).
- Normalization (reciprocal bcast + multiply) per (pair, s-block) on
  GPSIMD, recip broadcast via DRAM bounce.
- Output projection per s-block, overlapped with the next s-block's
  attention; result DMA'd straight from SBUF copies.
"""

import sys

sys.path.insert(0, "/opt/trn_rl_repo")

import numpy as np
import ml_dtypes

BF16 = ml_dtypes.bfloat16

B, S, E, H = 4, 2048, 1024, 16
D = E // H          # 64
HL = 8              # heads per core
NP = 4              # head pairs per core
N_CORES = 8
SB = 512            # s-block (psum bank) width

# DVE bit-exp: bf16 pattern of exp(l/8) ~= uint16(l * A + B)
A_EXP = float((2 ** 7 / np.log(2)) * 0.125)
B_EXP = float(127 * 128 - 7.0)

# which tt (0..15) of a (pair, s-block) slot compute exp on DVE
# (slot boundary tts 15/0/1 stay on ACT so DVE is free for the av release)
DVE_TT_STEADY = (2, 4, 5, 6, 8, 10, 12)
DVE_TT_SLOT0 = (8, 12)
DVE_TT_SLOT1 = (4, 6, 8, 10, 12)
PIPE_DELAY = 5      # attnV trails scores/exp by this many tt steps

_CACHE = {}


def build_nc():
    import concourse.mybir as mybir
    import concourse.tile as tile
    from concourse import bacc

    f32 = mybir.dt.float32
    bf16 = mybir.dt.bfloat16
    u16 = mybir.dt.uint16
    ADD = mybir.AluOpType.add
    MULT = mybir.AluOpType.mult

    n_tt = S // 128     # 16 t tiles
    n_sb = S // SB      # 4 s-blocks
    n_sc = S // 128     # 16 proj row tiles

    nc = bacc.Bacc(None)

    xp_d = nc.dram_tensor("xp", [NP, 128, S], bf16, kind="ExternalInput")
    wq_d = nc.dram_tensor("wq", [NP, 128, 128], bf16, kind="ExternalInput")
    wk_d = nc.dram_tensor("wk", [NP, 128, 128], bf16, kind="ExternalInput")
    wv_d = nc.dram_tensor("wv", [NP, 128, D + 1], bf16, kind="ExternalInput")
    bq_d = nc.dram_tensor("bq", [NP, 128, 1], f32, kind="ExternalInput")
    bk_d = nc.dram_tensor("bk", [NP, 128, 1], f32, kind="ExternalInput")
    bvb_d = nc.dram_tensor("bvb", [NP, 2, 4 * (D + 1)], bf16,
                           kind="ExternalInput")
    wot_d = nc.dram_tensor("wot", [HL * D, E], bf16, kind="ExternalInput")
    out_d = nc.dram_tensor("out", [S, E], f32, kind="ExternalOutput")
    recd = nc.dram_tensor("recd", [n_sb * NP, 2 * SB], f32)

    with tile.TileContext(nc) as tc:
        with (
            tc.tile_pool(name="xp", bufs=NP) as xp_pool,
            tc.tile_pool(name="w", bufs=3 * NP) as w_pool,
            tc.tile_pool(name="bias", bufs=4 * NP) as bias_pool,
            tc.tile_pool(name="qt", bufs=2 * NP) as qt_pool,
            tc.tile_pool(name="v", bufs=HL) as v_pool,
            tc.tile_pool(name="wot", bufs=4) as wot_pool,
            tc.tile_pool(name="et", bufs=8) as et_pool,
            tc.tile_pool(name="ct", bufs=NP) as ct_pool,
            tc.tile_pool(name="rel", bufs=6) as rel_pool,
            tc.tile_pool(name="bc", bufs=6) as bc_pool,
            tc.tile_pool(name="csrc", bufs=4) as cs_pool,
            tc.tile_pool(name="ctmp", bufs=2) as ctmp_pool,
            tc.tile_pool(name="osb", bufs=3) as osb_pool,
            tc.tile_pool(name="ps", bufs=3, space="PSUM") as ps_pool,
            tc.tile_pool(name="av", bufs=1, space="PSUM") as av_pool,
        ):
            # persistent tiles
            xps, qts, kts = [], [], []
            wqs, wks, wvs, bqs, bks, bvbs = [], [], [], [], [], []
            vss = []
            ct2s = []
            for p in range(NP):
                xps.append(xp_pool.tile([128, S], bf16, tag="xp",
                                        name=f"xp{p}"))
                qts.append(qt_pool.tile([128, S], bf16, tag="qt",
                                        name=f"qt{p}"))
                kts.append(qt_pool.tile([128, S], bf16, tag="qt",
                                        name=f"kt{p}"))
                wqs.append(w_pool.tile([128, 128], bf16, tag="w",
                                       name=f"wq{p}"))
                wks.append(w_pool.tile([128, 128], bf16, tag="w",
                                       name=f"wk{p}"))
                wvs.append(w_pool.tile([128, D + 1], bf16, tag="w",
                                       name=f"wv{p}"))
                bqs.append(bias_pool.tile([128, 1], f32, tag="bias",
                                          name=f"bq{p}"))
                bks.append(bias_pool.tile([128, 1], f32, tag="bias",
                                          name=f"bk{p}"))
                bvbs.append([bias_pool.tile([128, 4 * (D + 1)], bf16,
                                            tag="bias", name=f"bvb{p}_{h}")
                             for h in range(2)])
                vss.append([v_pool.tile([128, n_tt * (D + 1)], bf16, tag="v",
                                        name=f"vs{p}_{h}") for h in range(2)])
                ct2s.append(ct_pool.tile([128, S], bf16, tag="ct",
                                         name=f"ct{p}"))
            for p in range(NP):
                nc.sync.dma_start(out=xps[p][:, :], in_=xp_d[p])
                nc.scalar.dma_start(out=wqs[p][:, :], in_=wq_d[p])
                nc.scalar.dma_start(out=wks[p][:, :], in_=wk_d[p])
                nc.scalar.dma_start(out=wvs[p][:, :], in_=wv_d[p])
                nc.scalar.dma_start(out=bqs[p][:, :], in_=bq_d[p])
                nc.scalar.dma_start(out=bks[p][:, :], in_=bk_d[p])
                for h in range(2):
                    nc.scalar.dma_start(
                        out=bvbs[p][h][:, :],
                        in_=bvb_d[p, h].unsqueeze(0).broadcast_to(
                            (128, 4 * (D + 1))),
                    )
            wots = []
            for p in range(4):
                wt = wot_pool.tile([128, E], bf16, tag="wot", name=f"wot{p}")
                nc.sync.dma_start(out=wt[:, :],
                                  in_=wot_d[p * 128:(p + 1) * 128, :])
                wots.append(wt)

            def emit_qkv(p):
                # Q/K pair-packed projections, biases on the copy
                for which, wt, bias, dst in (
                    (0, wqs[p], bqs[p], qts[p]),
                    (1, wks[p], bks[p], kts[p]),
                ):
                    for half in range(2):
                        hsl = slice(half * 1024, (half + 1) * 1024)
                        psq = ps_pool.tile([128, 1024], f32, tag="ps",
                                           name=f"qk{p}_{which}_{half}")
                        for q4 in range(2):
                            qsl = slice(half * 1024 + q4 * 512,
                                        half * 1024 + (q4 + 1) * 512)
                            nc.tensor.matmul(psq[:, q4 * 512:(q4 + 1) * 512],
                                             wt[:, :], xps[p][:, qsl])
                        nc.vector.tensor_scalar_add(dst[:, hsl], psq[:, :],
                                                    bias[:, :])
                # V projections: row-tiled head pairs, 4 t-tiles per bank
                for tg in range(4):
                    psv = [ps_pool.tile([128, 1024], f32, tag="ps",
                                        name=f"v{p}_{tg}_{h}")
                           for h in range(2)]
                    for j in range(4):
                        tsl = slice((tg * 4 + j) * 128, (tg * 4 + j + 1) * 128)
                        csl = slice(j * (D + 1), (j + 1) * (D + 1))
                        nc.tensor.matmul(psv[0][:, csl], xps[p][0:64, tsl],
                                         wvs[p][0:64, :])
                        nc.tensor.matmul(psv[1][:, csl], xps[p][64:128, tsl],
                                         wvs[p][64:128, :])
                    gsl = slice(tg * 4 * (D + 1), (tg + 1) * 4 * (D + 1))
                    for h in range(2):
                        nc.vector.tensor_tensor(vss[p][h][:, gsl],
                                                psv[h][:, 0:4 * (D + 1)],
                                                bvbs[p][h][:, :], ADD)

            # slot state shared between the pipelined closures
            slot_av = {}

            def emit_step(p, sb, slot_idx, tt):
                """scores + exp for one tt of slot (p, sb)."""
                ssl = slice(sb * SB, (sb + 1) * SB)
                if slot_idx == 0:
                    dve_tt = DVE_TT_SLOT0
                elif slot_idx == 1:
                    dve_tt = DVE_TT_SLOT1
                else:
                    dve_tt = DVE_TT_STEADY
                tsl = slice(tt * 128, (tt + 1) * 128)
                ps = ps_pool.tile([128, 2 * SB], f32, tag="ps",
                                  name=f"sc{slot_idx}_{tt}")
                nc.tensor.matmul(ps[:, 0:SB], kts[p][0:64, tsl],
                                 qts[p][0:64, ssl])
                nc.tensor.matmul(ps[:, SB:2 * SB], kts[p][64:128, tsl],
                                 qts[p][64:128, ssl])
                et = et_pool.tile([128, 2 * SB], bf16, tag="et",
                                  name=f"et{slot_idx}_{tt}")
                if tt in dve_tt:
                    nc.vector.tensor_scalar(et[:, :].bitcast(u16),
                                            ps[:, :], A_EXP, B_EXP,
                                            MULT, ADD)
                else:
                    nc.scalar.activation(
                        et[:, :], ps[:, :],
                        mybir.ActivationFunctionType.Exp, scale=0.125)
                return et

            def emit_attnv(p, sb, slot_idx, tt, et):
                if tt == 0:
                    slot_av[slot_idx] = av_pool.tile(
                        [D + 1, 2 * SB], f32, tag="av", name=f"av{slot_idx}")
                av2 = slot_av[slot_idx]
                vsl = slice(tt * (D + 1), (tt + 1) * (D + 1))
                nc.tensor.matmul(av2[:, 0:SB], vss[p][0][:, vsl],
                                 et[:, 0:SB],
                                 start=(tt == 0), stop=(tt == n_tt - 1))
                nc.tensor.matmul(av2[:, SB:2 * SB], vss[p][1][:, vsl],
                                 et[:, SB:2 * SB],
                                 start=(tt == 0), stop=(tt == n_tt - 1))

            def emit_norm(p, sb, slot_idx):
                av2 = slot_av.pop(slot_idx)
                ssl = slice(sb * SB, (sb + 1) * SB)
                rel = [rel_pool.tile([D + 1, SB], bf16, tag="rel",
                                     name=f"rel{slot_idx}_{h}")
                       for h in range(2)]
                nc.vector.tensor_copy(rel[0][:, :], av2[:, 0:SB])
                nc.vector.tensor_copy(rel[1][:, :], av2[:, SB:2 * SB])
                cs = cs_pool.tile([128, 8], bf16, tag="csrc",
                                  name=f"cs{slot_idx}")
                nc.sync.dma_start(out=cs[:, 0:4], in_=rel[0][D:D + 1, :])
                nc.sync.dma_start(out=cs[:, 4:8], in_=rel[1][D:D + 1, :])
                rc = cs_pool.tile([128, 8], f32, tag="csrc",
                                  name=f"rc{slot_idx}")
                nc.vector.reciprocal(rc[:, :], cs[:, :])
                nc.sync.dma_start(out=recd[slot_idx, 0:SB], in_=rc[:, 0:4])
                nc.sync.dma_start(out=recd[slot_idx, SB:2 * SB],
                                  in_=rc[:, 4:8])
                bcs = []
                for h in range(2):
                    bc = bc_pool.tile([D, SB], f32, tag="bc",
                                      name=f"bc{slot_idx}_{h}")
                    nc.sync.dma_start(
                        out=bc[:, :],
                        in_=recd[slot_idx, h * SB:(h + 1) * SB]
                        .unsqueeze(0).broadcast_to((D, SB)))
                    bcs.append(bc)
                nc.gpsimd.tensor_tensor(ct2s[p][0:D, ssl], rel[0][0:D, :],
                                        bcs[0][:, :], MULT)
                ctmp = ctmp_pool.tile([D, SB], bf16, tag="ctmp",
                                      name=f"ctmp{slot_idx}")
                nc.gpsimd.tensor_tensor(ctmp[:, :], rel[1][0:D, :],
                                        bcs[1][:, :], MULT)
                nc.sync.dma_start(out=ct2s[p][D:2 * D, ssl], in_=ctmp[:, :])

            def emit_proj(sb):
                for sc in range(sb * 4, sb * 4 + 4):
                    csl = slice(sc * 128, (sc + 1) * 128)
                    pso = ps_pool.tile([128, 2 * SB], f32, tag="ps",
                                       name=f"pj{sc}")
                    for half in range(2):
                        hsl = slice(half * SB, (half + 1) * SB)
                        psl = pso[:, half * SB:(half + 1) * SB]
                        for p in range(NP):
                            nc.tensor.matmul(psl, ct2s[p][:, csl],
                                             wots[p][:, hsl],
                                             start=(p == 0), stop=(p == 3))
                    osb = osb_pool.tile([128, 2 * SB], f32, tag="osb",
                                        name=f"osb{sc}")
                    nc.vector.tensor_copy(osb[:, :], pso[:, :])
                    nc.scalar.dma_start(out=out_d[csl, :], in_=osb[:, :])

            # Global software pipeline: attnV (and trailing slot actions)
            # run PIPE_DELAY tt-steps behind scores/exp so the PE's
            # in-order queue never stalls waiting for an exp.
            from collections import deque
            deferred = deque()

            def run_deferred(n_keep):
                while len(deferred) > n_keep:
                    for fn in deferred.popleft():
                        fn()

            from functools import partial
            for sb in range(n_sb):
                for p in range(NP):
                    slot_idx = sb * NP + p
                    if sb == 0:
                        emit_qkv(p)
                    for tt in range(n_tt):
                        et = emit_step(p, sb, slot_idx, tt)
                        acts = [partial(emit_attnv, p, sb, slot_idx, tt, et)]
                        if tt == n_tt - 1:
                            acts.append(partial(emit_norm, p, sb, slot_idx))
                            if p == 0 and sb > 0:
                                acts.append(partial(emit_proj, sb - 1))
                        deferred.append(acts)
                        run_deferred(PIPE_DELAY)
            run_deferred(0)
            emit_proj(n_sb - 1)

    nc.compile()
    return nc


def prep_inputs(token_encodings, Wq, Wk, Wv, bq, bk, bv, Wo, bo):
    """Build per-core input maps. Core c = b*2+g."""
    x = np.asarray(token_encodings, dtype=np.float32)
    Wq = np.asarray(Wq, np.float32)
    Wk = np.asarray(Wk, np.float32)
    Wv = np.asarray(Wv, np.float32)
    bq = np.asarray(bq, np.float32)
    bk = np.asarray(bk, np.float32)
    bv = np.asarray(bv, np.float32)
    Wo = np.asarray(Wo, np.float32)
    maps = []
    for c in range(N_CORES):
        b, g = divmod(c, 2)
        xt_full = np.ascontiguousarray(x[b].T)  # (E, S)
        xp = np.zeros((NP, 128, S), dtype=BF16)
        wq_a = np.zeros((NP, 128, 128), dtype=BF16)
        wk_a = np.zeros((NP, 128, 128), dtype=BF16)
        wv_a = np.zeros((NP, 128, D + 1), dtype=BF16)
        bq_a = np.zeros((NP, 128, 1), dtype=np.float32)
        bk_a = np.zeros((NP, 128, 1), dtype=np.float32)
        bvb_a = np.zeros((NP, 2, 4 * (D + 1)), dtype=BF16)
        for p in range(NP):
            for h in range(2):
                hh = g * HL + 2 * p + h
                rsl = slice(h * 64, (h + 1) * 64)
                xp[p, rsl] = xt_full[hh * D:(hh + 1) * D].astype(BF16)
                wq_a[p, rsl, rsl] = Wq[hh].astype(BF16)
                wk_a[p, rsl, rsl] = Wk[hh].astype(BF16)
                wv_a[p, rsl, 0:D] = Wv[hh].astype(BF16)
                bq_a[p, rsl, 0] = bq[hh]
                bk_a[p, rsl, 0] = bk[hh]
                one_bv = np.concatenate([bv[hh], [1.0]]).astype(BF16)
                bvb_a[p, h] = np.tile(one_bv, 4)
        wot = np.ascontiguousarray(
            Wo[:, g * 512:(g + 1) * 512].T).astype(BF16)
        maps.append({"xp": xp, "wq": wq_a, "wk": wk_a, "wv": wv_a,
                     "bq": bq_a, "bk": bk_a, "bvb": bvb_a, "wot": wot})
    return maps


def kernel(**inputs):
    from concourse.bass_utils import run_bass_kernel_spmd

    if "nc" not in _CACHE:
        _CACHE["nc"] = build_nc()
    nc = _CACHE["nc"]
    in_maps = prep_inputs(**inputs)
    res = run_bass_kernel_spmd(nc, in_maps, list(range(N_CORES)))
    bo_f = np.asarray(inputs["bo"], np.float32)
    out = np.empty((B, S, E), dtype=np.float32)
    for b in range(B):
        out[b] = res.results[2 * b]["out"] + res.results[2 * b + 1]["out"] + bo_f
    return out


# revision 14
# speedup vs baseline: 1.1058x; 1.0774x over previous
"""Trainium2 Bass kernel: multi-head attention (B=4, S=2048, E=1024, H=16, D=64).

Sharding: 8 cores = 4 batches x 2 head-groups (8 heads each). Each core
computes attention for its (batch, 8-head group) and a partial output
projection over its 512 channels; the host sums the two partials per batch
and adds the output bias.

v2 design (vs v1 baseline at ~355us):
- Heads processed in pairs: head A occupies SBUF partitions 0-63, head B
  64-127.  QK^T score matmuls are K=64 row-tiled pairs (tile_position
  (0,0)/(64,0)) that execute CONCURRENTLY on the PE array -> ~1.7x faster
  scores (measured in micro-benchmark).
- Q/K projections pair-packed with block-diagonal weights (K=128 full
  array); biases added on the PSUM->SBUF copy (DVE tensor_scalar_add).
- exp() split between the Scalar engine (table exp, ~9/16 of tiles) and
  the Vector engine (Schraudolph-style bit-trick exp producing bf16 via
  uint16 bitcast, ~7/16) -- the Scalar engine alone (306us busy) was the
  baseline co-bottleneck.
- attnV: V_aug stationary (65 cols, ones column -> softmax denominator
  in row 64 of the accumulator).
- Normalization (reciprocal bcast + multiply) per (pair, s-block) on
  GPSIMD, recip broadcast via DRAM bounce.
- Output projection per s-block, overlapped with the next s-block's
  attention; result DMA'd straight from SBUF copies.
"""

import sys

sys.path.insert(0, "/opt/trn_rl_repo")

import numpy as np
import ml_dtypes

BF16 = ml_dtypes.bfloat16

B, S, E, H = 4, 2048, 1024, 16
D = E // H          # 64
HL = 8              # heads per core
NP = 4              # head pairs per core
N_CORES = 8
SB = 512            # s-block (psum bank) width

# DVE bit-exp: bf16 pattern of exp(l/8) ~= uint16(l * A + B)
A_EXP = float((2 ** 7 / np.log(2)) * 0.125)
B_EXP = float(127 * 128 - 7.0)

# which tt (0..15) of a (pair, s-block) slot compute exp on DVE
# (slot boundary tts 15/0/1 stay on ACT so DVE is free for the av release)
DVE_TT_STEADY = (2, 4, 5, 6, 8, 10, 12)
DVE_TT_SLOT0 = (8, 12)
DVE_TT_SLOT1 = (4, 6, 8, 10, 12)
PIPE_DELAY = 4      # attnV trails scores/exp by this many tt steps

_CACHE = {}


def build_nc():
    import concourse.mybir as mybir
    import concourse.tile as tile
    from concourse import bacc

    f32 = mybir.dt.float32
    bf16 = mybir.dt.bfloat16
    u16 = mybir.dt.uint16
    ADD = mybir.AluOpType.add
    MULT = mybir.AluOpType.mult

    n_tt = S // 128     # 16 t tiles
    n_sb = S // SB      # 4 s-blocks
    n_sc = S // 128     # 16 proj row tiles

    nc = bacc.Bacc(None)

    xp_d = nc.dram_tensor("xp", [NP, 128, S], bf16, kind="ExternalInput")
    WPACK = 128 + 128 + (D + 1) + 2 * 4 * (D + 1)   # wq|wk|wv|bvbA|bvbB
    wp_d = nc.dram_tensor("wp", [NP, 128, WPACK], bf16, kind="ExternalInput")
    bp_d = nc.dram_tensor("bp", [NP, 128, 2], f32, kind="ExternalInput")
    wot_d = nc.dram_tensor("wot", [HL * D, E], bf16, kind="ExternalInput")
    out_d = nc.dram_tensor("out", [S, E], f32, kind="ExternalOutput")
    recd = nc.dram_tensor("recd", [n_sb * NP, 2 * SB], f32)

    with tile.TileContext(nc) as tc:
        with (
            tc.tile_pool(name="xp", bufs=NP) as xp_pool,
            tc.tile_pool(name="w", bufs=3 * NP) as w_pool,
            tc.tile_pool(name="bias", bufs=4 * NP) as bias_pool,
            tc.tile_pool(name="qt", bufs=2 * NP) as qt_pool,
            tc.tile_pool(name="v", bufs=HL) as v_pool,
            tc.tile_pool(name="wot", bufs=4) as wot_pool,
            tc.tile_pool(name="et", bufs=7) as et_pool,
            tc.tile_pool(name="ct", bufs=NP) as ct_pool,
            tc.tile_pool(name="rel", bufs=6) as rel_pool,
            tc.tile_pool(name="bc", bufs=6) as bc_pool,
            tc.tile_pool(name="csrc", bufs=4) as cs_pool,
            tc.tile_pool(name="ctmp", bufs=2) as ctmp_pool,
            tc.tile_pool(name="osb", bufs=3) as osb_pool,
            tc.tile_pool(name="ps", bufs=3, space="PSUM") as ps_pool,
            tc.tile_pool(name="av", bufs=1, space="PSUM") as av_pool,
        ):
            # persistent tiles
            xps, qts, kts = [], [], []
            wqs, wks, wvs, bqs, bks, bvbs = [], [], [], [], [], []
            wps, bps = [], []
            vss = []
            ct2s = []
            for p in range(NP):
                xps.append(xp_pool.tile([128, S], bf16, tag="xp",
                                        name=f"xp{p}"))
                qts.append(qt_pool.tile([128, S], bf16, tag="qt",
                                        name=f"qt{p}"))
                kts.append(qt_pool.tile([128, S], bf16, tag="qt",
                                        name=f"kt{p}"))
                wp = w_pool.tile([128, WPACK], bf16, tag="w",
                                 name=f"wp{p}")
                wps.append(wp)
                wqs.append(wp[:, 0:128])
                wks.append(wp[:, 128:256])
                wvs.append(wp[:, 256:256 + (D + 1)])
                b0 = 256 + (D + 1)
                bvbs.append([wp[:, b0 + h * 4 * (D + 1):
                                b0 + (h + 1) * 4 * (D + 1)]
                             for h in range(2)])
                bp = bias_pool.tile([128, 2], f32, tag="bias",
                                    name=f"bp{p}")
                bps.append(bp)
                bqs.append(bp[:, 0:1])
                bks.append(bp[:, 1:2])
                vss.append([v_pool.tile([128, n_tt * (D + 1)], bf16, tag="v",
                                        name=f"vs{p}_{h}") for h in range(2)])
                ct2s.append(ct_pool.tile([128, S], bf16, tag="ct",
                                         name=f"ct{p}"))
            for p in range(NP):
                nc.sync.dma_start(out=xps[p][:, :], in_=xp_d[p])
                nc.sync.dma_start(out=wps[p][:, :], in_=wp_d[p])
                nc.sync.dma_start(out=bps[p][:, :], in_=bp_d[p])
            wots = []
            for p in range(4):
                wt = wot_pool.tile([128, E], bf16, tag="wot", name=f"wot{p}")
                nc.sync.dma_start(out=wt[:, :],
                                  in_=wot_d[p * 128:(p + 1) * 128, :])
                wots.append(wt)

            def emit_qkv(p):
                # Q/K pair-packed projections, biases on the copy
                for which, wt, bias, dst in (
                    (0, wqs[p], bqs[p], qts[p]),
                    (1, wks[p], bks[p], kts[p]),
                ):
                    for half in range(2):
                        hsl = slice(half * 1024, (half + 1) * 1024)
                        psq = ps_pool.tile([128, 1024], f32, tag="ps",
                                           name=f"qk{p}_{which}_{half}")
                        for q4 in range(2):
                            qsl = slice(half * 1024 + q4 * 512,
                                        half * 1024 + (q4 + 1) * 512)
                            nc.tensor.matmul(psq[:, q4 * 512:(q4 + 1) * 512],
                                             wt[:, :], xps[p][:, qsl])
                        nc.vector.tensor_scalar_add(dst[:, hsl], psq[:, :],
                                                    bias[:, :])
                # V projections: row-tiled head pairs, 4 t-tiles per bank
                for tg in range(4):
                    psv = [ps_pool.tile([128, 1024], f32, tag="ps",
                                        name=f"v{p}_{tg}_{h}")
                           for h in range(2)]
                    for j in range(4):
                        tsl = slice((tg * 4 + j) * 128, (tg * 4 + j + 1) * 128)
                        csl = slice(j * (D + 1), (j + 1) * (D + 1))
                        nc.tensor.matmul(psv[0][:, csl], xps[p][0:64, tsl],
                                         wvs[p][0:64, :])
                        nc.tensor.matmul(psv[1][:, csl], xps[p][64:128, tsl],
                                         wvs[p][64:128, :])
                    gsl = slice(tg * 4 * (D + 1), (tg + 1) * 4 * (D + 1))
                    for h in range(2):
                        nc.vector.tensor_tensor(vss[p][h][:, gsl],
                                                psv[h][:, 0:4 * (D + 1)],
                                                bvbs[p][h][:, :], ADD)

            # slot state shared between the pipelined closures
            slot_av = {}

            def emit_step(p, sb, slot_idx, tt):
                """scores + exp for one tt of slot (p, sb)."""
                ssl = slice(sb * SB, (sb + 1) * SB)
                if slot_idx == 0:
                    dve_tt = DVE_TT_SLOT0
                elif slot_idx == 1:
                    dve_tt = DVE_TT_SLOT1
                else:
                    dve_tt = DVE_TT_STEADY
                tsl = slice(tt * 128, (tt + 1) * 128)
                ps = ps_pool.tile([128, 2 * SB], f32, tag="ps",
                                  name=f"sc{slot_idx}_{tt}")
                nc.tensor.matmul(ps[:, 0:SB], kts[p][0:64, tsl],
                                 qts[p][0:64, ssl])
                nc.tensor.matmul(ps[:, SB:2 * SB], kts[p][64:128, tsl],
                                 qts[p][64:128, ssl])
                et = et_pool.tile([128, 2 * SB], bf16, tag="et",
                                  name=f"et{slot_idx}_{tt}")
                if tt in dve_tt:
                    nc.vector.tensor_scalar(et[:, :].bitcast(u16),
                                            ps[:, :], A_EXP, B_EXP,
                                            MULT, ADD)
                else:
                    nc.scalar.activation(
                        et[:, :], ps[:, :],
                        mybir.ActivationFunctionType.Exp, scale=0.125)
                return et

            def emit_attnv(p, sb, slot_idx, tt, et):
                if tt == 0:
                    slot_av[slot_idx] = av_pool.tile(
                        [D + 1, 2 * SB], f32, tag="av", name=f"av{slot_idx}")
                av2 = slot_av[slot_idx]
                vsl = slice(tt * (D + 1), (tt + 1) * (D + 1))
                nc.tensor.matmul(av2[:, 0:SB], vss[p][0][:, vsl],
                                 et[:, 0:SB],
                                 start=(tt == 0), stop=(tt == n_tt - 1))
                nc.tensor.matmul(av2[:, SB:2 * SB], vss[p][1][:, vsl],
                                 et[:, SB:2 * SB],
                                 start=(tt == 0), stop=(tt == n_tt - 1))

            def emit_norm(p, sb, slot_idx):
                av2 = slot_av.pop(slot_idx)
                ssl = slice(sb * SB, (sb + 1) * SB)
                rel = [rel_pool.tile([D + 1, SB], bf16, tag="rel",
                                     name=f"rel{slot_idx}_{h}")
                       for h in range(2)]
                nc.vector.tensor_copy(rel[0][:, :], av2[:, 0:SB])
                nc.vector.tensor_copy(rel[1][:, :], av2[:, SB:2 * SB])
                cs = cs_pool.tile([128, 8], bf16, tag="csrc",
                                  name=f"cs{slot_idx}")
                nc.sync.dma_start(out=cs[:, 0:4], in_=rel[0][D:D + 1, :])
                nc.sync.dma_start(out=cs[:, 4:8], in_=rel[1][D:D + 1, :])
                rc = cs_pool.tile([128, 8], f32, tag="csrc",
                                  name=f"rc{slot_idx}")
                nc.vector.reciprocal(rc[:, :], cs[:, :])
                nc.sync.dma_start(out=recd[slot_idx, 0:SB], in_=rc[:, 0:4])
                nc.sync.dma_start(out=recd[slot_idx, SB:2 * SB],
                                  in_=rc[:, 4:8])
                bcs = []
                for h in range(2):
                    bc = bc_pool.tile([D, SB], f32, tag="bc",
                                      name=f"bc{slot_idx}_{h}")
                    nc.sync.dma_start(
                        out=bc[:, :],
                        in_=recd[slot_idx, h * SB:(h + 1) * SB]
                        .unsqueeze(0).broadcast_to((D, SB)))
                    bcs.append(bc)
                nc.gpsimd.tensor_tensor(ct2s[p][0:D, ssl], rel[0][0:D, :],
                                        bcs[0][:, :], MULT)
                ctmp = ctmp_pool.tile([D, SB], bf16, tag="ctmp",
                                      name=f"ctmp{slot_idx}")
                nc.gpsimd.tensor_tensor(ctmp[:, :], rel[1][0:D, :],
                                        bcs[1][:, :], MULT)
                nc.sync.dma_start(out=ct2s[p][D:2 * D, ssl], in_=ctmp[:, :])

            def emit_proj(sb):
                for sc in range(sb * 4, sb * 4 + 4):
                    csl = slice(sc * 128, (sc + 1) * 128)
                    pso = ps_pool.tile([128, 2 * SB], f32, tag="ps",
                                       name=f"pj{sc}")
                    for half in range(2):
                        hsl = slice(half * SB, (half + 1) * SB)
                        psl = pso[:, half * SB:(half + 1) * SB]
                        for p in range(NP):
                            nc.tensor.matmul(psl, ct2s[p][:, csl],
                                             wots[p][:, hsl],
                                             start=(p == 0), stop=(p == 3))
                    osb = osb_pool.tile([128, 2 * SB], f32, tag="osb",
                                        name=f"osb{sc}")
                    nc.vector.tensor_copy(osb[:, :], pso[:, :])
                    nc.sync.dma_start(out=out_d[csl, :], in_=osb[:, :])

            # Global software pipeline: attnV (and trailing slot actions)
            # run PIPE_DELAY tt-steps behind scores/exp so the PE's
            # in-order queue never stalls waiting for an exp.
            from collections import deque
            deferred = deque()

            def run_deferred(n_keep):
                while len(deferred) > n_keep:
                    for fn in deferred.popleft():
                        fn()

            from functools import partial
            for sb in range(n_sb):
                for p in range(NP):
                    slot_idx = sb * NP + p
                    if sb == 0:
                        emit_qkv(p)
                    for tt in range(n_tt):
                        et = emit_step(p, sb, slot_idx, tt)
                        acts = [partial(emit_attnv, p, sb, slot_idx, tt, et)]
                        if tt == n_tt - 1:
                            acts.append(partial(emit_norm, p, sb, slot_idx))
                            if p == 0 and sb > 0:
                                acts.append(partial(emit_proj, sb - 1))
                        deferred.append(acts)
                        run_deferred(PIPE_DELAY)
            run_deferred(0)
            emit_proj(n_sb - 1)

    nc.compile()
    return nc


def prep_inputs(token_encodings, Wq, Wk, Wv, bq, bk, bv, Wo, bo):
    """Build per-core input maps. Core c = b*2+g."""
    x = np.asarray(token_encodings, dtype=np.float32)
    Wq = np.asarray(Wq, np.float32)
    Wk = np.asarray(Wk, np.float32)
    Wv = np.asarray(Wv, np.float32)
    bq = np.asarray(bq, np.float32)
    bk = np.asarray(bk, np.float32)
    bv = np.asarray(bv, np.float32)
    Wo = np.asarray(Wo, np.float32)
    maps = []
    for c in range(N_CORES):
        b, g = divmod(c, 2)
        xt_full = np.ascontiguousarray(x[b].T)  # (E, S)
        WPACK = 128 + 128 + (D + 1) + 2 * 4 * (D + 1)
        xp = np.zeros((NP, 128, S), dtype=BF16)
        wp_a = np.zeros((NP, 128, WPACK), dtype=BF16)
        bp_a = np.zeros((NP, 128, 2), dtype=np.float32)
        b0 = 256 + (D + 1)
        for p in range(NP):
            for h in range(2):
                hh = g * HL + 2 * p + h
                rsl = slice(h * 64, (h + 1) * 64)
                xp[p, rsl] = xt_full[hh * D:(hh + 1) * D].astype(BF16)
                wp_a[p, rsl, rsl] = Wq[hh].astype(BF16)
                wp_a[p, rsl, 128 + h * 64:128 + (h + 1) * 64] = \
                    Wk[hh].astype(BF16)
                wp_a[p, rsl, 256:256 + D] = Wv[hh].astype(BF16)
                bp_a[p, rsl, 0] = bq[hh]
                bp_a[p, rsl, 1] = bk[hh]
                one_bv = np.concatenate([bv[hh], [1.0]]).astype(BF16)
                bsl = slice(b0 + h * 4 * (D + 1), b0 + (h + 1) * 4 * (D + 1))
                wp_a[p, :, bsl] = np.tile(one_bv, 4)[None, :]
        wot = np.ascontiguousarray(
            Wo[:, g * 512:(g + 1) * 512].T).astype(BF16)
        maps.append({"xp": xp, "wp": wp_a, "bp": bp_a, "wot": wot})
    return maps


def kernel(**inputs):
    from concourse.bass_utils import run_bass_kernel_spmd

    if "nc" not in _CACHE:
        _CACHE["nc"] = build_nc()
    nc = _CACHE["nc"]
    in_maps = prep_inputs(**inputs)
    res = run_bass_kernel_spmd(nc, in_maps, list(range(N_CORES)))
    bo_f = np.asarray(inputs["bo"], np.float32)
    out = np.empty((B, S, E), dtype=np.float32)
    for b in range(B):
        out[b] = res.results[2 * b]["out"] + res.results[2 * b + 1]["out"] + bo_f
    return out
